# revision 1
# baseline (speedup 1.0000x reference)
"""IsometricLoss on 8 Trainium2 NeuronCores (data-parallel over N).

loss = sum(r * max(||x||^2 + ||mu||^2 - 2 x.mu, 0)) / N

For random-normal X/mus the squared distances are ~2*D >> fp32 noise, so the
max(.,0) clamp never binds and the loss decomposes exactly:

  loss*N = sum_{m,d} S1[m,d] * (-2 mus[m,d])   (cross term)
         + sum_m    q[m]                        (||x||^2 term)
         + sum_m    rc[m] * mu2[m]              (||mu||^2 term)

with S1 = r^T X  [M,D],  q = r^T x2  [M],  rc = r^T 1  [M],
x2[n] = ||X[n]||^2, mu2[m] = ||mus[m]||^2.

Each core streams its N/8 shard of X and r exactly once (memory-bound):
per 128-row chunk a single fp32 PE matmul accumulates
    psum[128, 130] += r_chunk^T @ [x_chunk | x2_col | ones]
into one PSUM bank.  ScalarE squares X, VectorE does the segmented
row-norm reduce, and a tiny tail contracts psum against
Mext = [-2*mus | 1 | mu2] to a per-core scalar.  Host sums the 8 partials.
"""

from contextlib import ExitStack

import numpy as np

import concourse.bass as bass
import concourse.tile as tile
from concourse import bacc, mybir
from concourse.bass_utils import run_bass_kernel_spmd

N, M, D = 131072, 128, 128
NCORES = 8
NSHARD = N // NCORES          # 16384 rows per core
P = 128                       # chunk rows == SBUF partitions
CHUNKS = NSHARD // P          # 128 chunks per core
K = 32                        # chunks per outer iteration
ITERS = CHUNKS // K           # 4
W = D + 2                     # per-chunk moving cols: [x(128) | x2(1) | 1(1)]

F32 = mybir.dt.float32
_cache: dict[str, object] = {}


def _build():
    nc = bacc.Bacc(
        "TRN2",
        target_bir_lowering=False,
        debug=False,
        num_devices=NCORES,
    )

    x_d = nc.dram_tensor("X", [NSHARD, D], F32, kind="ExternalInput").ap()
    r_d = nc.dram_tensor("r", [NSHARD, M], F32, kind="ExternalInput").ap()
    mus_d = nc.dram_tensor("mus", [M, D], F32, kind="ExternalInput").ap()
    out_d = nc.dram_tensor("out", [1, 1], F32, kind="ExternalOutput").ap()

    # [NSHARD, D] -> [p, chunk, d]: row n = chunk*128 + p
    x_r = x_d.rearrange("(c p) d -> p c d", p=P)
    r_r = r_d.rearrange("(c p) m -> p c m", p=P)

    with tile.TileContext(nc) as tc, ExitStack() as ctx:
        singles = ctx.enter_context(tc.tile_pool(name="singles", bufs=1))
        big = ctx.enter_context(tc.tile_pool(name="big", bufs=3))
        psum_pool = ctx.enter_context(tc.tile_pool(name="psum", bufs=1, space="PSUM"))

        # Mext = [-2*mus | 1 | mu2], contracted against psum at the end.
        mus_t = singles.tile([M, D], F32)
        nc.sync.dma_start(mus_t[:], mus_d[:])
        mext = singles.tile([M, W], F32)
        nc.scalar.mul(mext[:, 0:D], mus_t[:], -2.0)
        nc.vector.memset(mext[:, D : D + 1], 1.0)
        mus_sq = singles.tile([M, D], F32)
        nc.vector.tensor_mul(mus_sq[:], mus_t[:], mus_t[:])
        nc.vector.reduce_sum(
            mext[:, D + 1 : D + 2], mus_sq[:], axis=mybir.AxisListType.X
        )

        ones_col = singles.tile([P, 1], F32)
        nc.vector.memset(ones_col[:], 1.0)

        psum_acc = psum_pool.tile([M, W], F32)

        for t in range(ITERS):
            comb = big.tile([P, K, W], F32, tag="comb")
            rwide = big.tile([P, K, M], F32, tag="rwide")
            sq = big.tile([P, K, D], F32, tag="sq")

            nc.sync.dma_start(comb[:, :, 0:D], x_r[:, t * K : (t + 1) * K, :])
            nc.sync.dma_start(rwide[:], r_r[:, t * K : (t + 1) * K, :])

            nc.scalar.activation(
                sq[:], comb[:, :, 0:D], mybir.ActivationFunctionType.Square
            )
            nc.vector.reduce_sum(
                comb[:, :, D : D + 1], sq[:], axis=mybir.AxisListType.X
            )
            nc.vector.memset(comb[:, :, D + 1 : D + 2], 1.0)

            for j in range(K):
                idx = t * K + j
                nc.tensor.matmul(
                    psum_acc[:],
                    rwide[:, j, :],
                    comb[:, j, :],
                    start=(idx == 0),
                    stop=(idx == CHUNKS - 1),
                )

        # Tail: scalar partial = sum(psum_acc * Mext)
        prod = singles.tile([M, W], F32)
        nc.vector.tensor_mul(prod[:], psum_acc[:], mext[:])
        u = singles.tile([M, 1], F32)
        nc.vector.reduce_sum(u[:], prod[:], axis=mybir.AxisListType.X)
        psum_s = psum_pool.tile([1, 1], F32)
        nc.tensor.matmul(psum_s[:], u[:], ones_col[:], start=True, stop=True)
        res = singles.tile([1, 1], F32)
        nc.vector.tensor_copy(res[:], psum_s[:])
        nc.sync.dma_start(out_d[:], res[:])

    nc.compile()
    return nc


def _get_nc():
    if "nc" not in _cache:
        _cache["nc"] = _build()
    return _cache["nc"]


def _run(X, r, mus, **spmd_kwargs):
    X = np.ascontiguousarray(np.asarray(X, dtype=np.float32))
    r = np.ascontiguousarray(np.asarray(r, dtype=np.float32))
    mus = np.ascontiguousarray(np.asarray(mus, dtype=np.float32))
    assert X.shape == (N, D) and r.shape == (N, M) and mus.shape == (M, D)

    nc = _get_nc()
    in_maps = [
        {
            "X": X[c * NSHARD : (c + 1) * NSHARD],
            "r": r[c * NSHARD : (c + 1) * NSHARD],
            "mus": mus,
        }
        for c in range(NCORES)
    ]
    return run_bass_kernel_spmd(nc, in_maps, core_ids=list(range(NCORES)), **spmd_kwargs)


def kernel(X, r, mus):
    out = _run(X, r, mus)
    total = sum(float(out.results[c]["out"][0, 0]) for c in range(NCORES))
    return np.float32(total / N)


# revision 2
# speedup vs baseline: 1.1961x; 1.1961x over previous
"""IsometricLoss on 8 Trainium2 NeuronCores (data-parallel over N).

loss = sum(r * max(||x||^2 + ||mu||^2 - 2 x.mu, 0)) / N

For random-normal X/mus the squared distances are ~2*D >> fp32 noise, so the
max(.,0) clamp never binds and the loss decomposes exactly:

  loss*N = sum_{m,d} S1[m,d] * (-2 mus[m,d])     (cross term)
         + sum_m    (q_hi[m] + q_lo[m])          (||x||^2 term)
         + sum_m    rc[m] * mu2[m]               (||mu||^2 term)

with S1 = r^T X  [M,D],  q = r^T x2  [M],  rc = r^T 1  [M],
x2[n] = ||X[n]||^2 (split into bf16 hi+lo columns), mu2[m] = ||mus[m]||^2.

Each core streams its N/8 shard of X and r exactly once (memory-bound).
Rows are packed 8-per-partition so each DMA descriptor moves 4 KiB
(512B descriptors only reach ~82% of HBM bandwidth).  Per 128-row group a
single bf16 PE matmul accumulates

    psum[128, 131] += r_grp^T @ [x_bf16 | x2_hi | x2_lo | ones]

into one PSUM bank (fp32 fp32-accumulate).  bf16 is safe here: the loss is
linear in r and bilinear in X, so round-to-nearest input errors (~2^-9
relative) cancel statistically over 16M products (~1e-7 net), while the
large ||x||^2 term rides the exact fp32 hi+lo pair.  ScalarE squares and
casts X, VectorE casts r and does the segmented row-norm reduce, and a tiny
tail contracts psum against Mext = [-2*mus | 1 | 1 | mu2] to a per-core
scalar.  Host sums the 8 partials.
"""

from contextlib import ExitStack

import numpy as np

import concourse.bass as bass
import concourse.tile as tile
from concourse import bacc, mybir
from concourse.bass_utils import run_bass_kernel_spmd

N, M, D = 131072, 128, 128
NCORES = 8
NSHARD = N // NCORES          # 16384 rows per core
P = 128                       # SBUF partitions
G = 8                         # rows packed per partition (4 KiB DMA runs)
S = 2                         # row-groups of 128*G per outer iteration
ITERS = NSHARD // (S * P * G)  # 8
W = D + 3                     # moving cols per group: [x(128) | hi | lo | 1]

F32 = mybir.dt.float32
BF16 = mybir.dt.bfloat16
_cache: dict[str, object] = {}


def _build():
    nc = bacc.Bacc(
        "TRN2",
        target_bir_lowering=False,
        debug=False,
        num_devices=NCORES,
    )

    x_d = nc.dram_tensor("X", [NSHARD, D], F32, kind="ExternalInput").ap()
    r_d = nc.dram_tensor("r", [NSHARD, M], F32, kind="ExternalInput").ap()
    mus_d = nc.dram_tensor("mus", [M, D], F32, kind="ExternalInput").ap()
    out_d = nc.dram_tensor("out", [1, 1], F32, kind="ExternalOutput").ap()

    # row = ((i*S + s)*P + p)*G + g  ->  AP dims [i, p, s, g, d]
    x_r = x_d.rearrange("(i s p g) d -> i p s g d", s=S, p=P, g=G)
    r_r = r_d.rearrange("(i s p g) m -> i p s g m", s=S, p=P, g=G)

    with tile.TileContext(nc) as tc, ExitStack() as ctx:
        singles = ctx.enter_context(tc.tile_pool(name="singles", bufs=1))
        big = ctx.enter_context(tc.tile_pool(name="big", bufs=3))
        psum_pool = ctx.enter_context(tc.tile_pool(name="psum", bufs=1, space="PSUM"))

        # Mext = [-2*mus | 1 | 1 | mu2], contracted against psum at the end.
        mus_t = singles.tile([M, D], F32)
        nc.sync.dma_start(mus_t[:], mus_d[:])
        mext = singles.tile([M, W], F32)
        nc.scalar.mul(mext[:, 0:D], mus_t[:], -2.0)
        nc.vector.memset(mext[:, D : D + 2], 1.0)
        mus_sq = singles.tile([M, D], F32)
        nc.vector.tensor_mul(mus_sq[:], mus_t[:], mus_t[:])
        nc.vector.reduce_sum(
            mext[:, D + 2 : D + 3], mus_sq[:], axis=mybir.AxisListType.X
        )

        ones_col = singles.tile([P, 1], F32)
        nc.vector.memset(ones_col[:], 1.0)

        psum_acc = psum_pool.tile([M, W], F32)

        for t in range(ITERS):
            x_f32 = big.tile([P, S, G, D], F32, tag="x_f32")
            r_f32 = big.tile([P, S, G, M], F32, tag="r_f32")
            sq = big.tile([P, S, G, D], F32, tag="sq")
            comb = big.tile([P, S, G, W], BF16, tag="comb")
            r_bf = big.tile([P, S, G, M], BF16, tag="r_bf")
            x2b = big.tile([P, S, G, 1], F32, tag="x2b")
            hi32 = big.tile([P, S, G, 1], F32, tag="hi32")
            lo32 = big.tile([P, S, G, 1], F32, tag="lo32")

            nc.sync.dma_start(x_f32[:], x_r[t])
            nc.sync.dma_start(r_f32[:], r_r[t])

            # bf16 casts of the matmul operands
            nc.scalar.activation(
                comb[:, :, :, 0:D], x_f32[:], mybir.ActivationFunctionType.Copy
            )
            nc.vector.tensor_copy(r_bf[:], r_f32[:])

            # exact row norms x2 = sum_d x^2, split into bf16 hi+lo columns
            nc.scalar.activation(
                sq[:], x_f32[:], mybir.ActivationFunctionType.Square
            )
            nc.vector.reduce_sum(x2b[:], sq[:], axis=mybir.AxisListType.X)
            nc.vector.tensor_copy(comb[:, :, :, D : D + 1], x2b[:])   # hi (rn)
            nc.vector.tensor_copy(hi32[:], comb[:, :, :, D : D + 1])  # back to f32
            nc.vector.tensor_sub(lo32[:], x2b[:], hi32[:])
            nc.vector.tensor_copy(comb[:, :, :, D + 1 : D + 2], lo32[:])
            nc.vector.memset(comb[:, :, :, D + 2 : D + 3], 1.0)

            for s in range(S):
                for g in range(G):
                    idx = (t * S + s) * G + g
                    nc.tensor.matmul(
                        psum_acc[:],
                        r_bf[:, s, g, :],
                        comb[:, s, g, :],
                        start=(idx == 0),
                        stop=(idx == ITERS * S * G - 1),
                    )

        # Tail: scalar partial = sum(psum_acc * Mext)
        prod = singles.tile([M, W], F32)
        nc.vector.tensor_mul(prod[:], psum_acc[:], mext[:])
        u = singles.tile([M, 1], F32)
        nc.vector.reduce_sum(u[:], prod[:], axis=mybir.AxisListType.X)
        psum_s = psum_pool.tile([1, 1], F32)
        nc.tensor.matmul(psum_s[:], u[:], ones_col[:], start=True, stop=True)
        res = singles.tile([1, 1], F32)
        nc.vector.tensor_copy(res[:], psum_s[:])
        nc.sync.dma_start(out_d[:], res[:])

    nc.compile()
    return nc


def _get_nc():
    if "nc" not in _cache:
        _cache["nc"] = _build()
    return _cache["nc"]


def _run(X, r, mus, **spmd_kwargs):
    X = np.ascontiguousarray(np.asarray(X, dtype=np.float32))
    r = np.ascontiguousarray(np.asarray(r, dtype=np.float32))
    mus = np.ascontiguousarray(np.asarray(mus, dtype=np.float32))
    assert X.shape == (N, D) and r.shape == (N, M) and mus.shape == (M, D)

    nc = _get_nc()
    in_maps = [
        {
            "X": X[c * NSHARD : (c + 1) * NSHARD],
            "r": r[c * NSHARD : (c + 1) * NSHARD],
            "mus": mus,
        }
        for c in range(NCORES)
    ]
    return run_bass_kernel_spmd(nc, in_maps, core_ids=list(range(NCORES)), **spmd_kwargs)


def kernel(X, r, mus):
    out = _run(X, r, mus)
    total = sum(float(out.results[c]["out"][0, 0]) for c in range(NCORES))
    return np.float32(total / N)


# revision 3
# speedup vs baseline: 1.3710x; 1.1463x over previous
"""IsometricLoss on 8 Trainium2 NeuronCores (data-parallel over N).

loss = sum(r * max(||x||^2 + ||mu||^2 - 2 x.mu, 0)) / N

For random-normal X/mus the squared distances are ~2*D >> fp32 noise, so the
max(.,0) clamp never binds and the loss decomposes exactly:

  loss*N = sum_{m,d} S1[m,d] * (-2 mus[m,d])     (cross term,  S1 = r^T X)
         + sum_{m,d} S2[m,d]                     (||x||^2 term, S2 = r^T X.^2)
         + sum_m    rc[m] * mu2[m]               (||mu||^2 term, rc = r^T 1)

Each core streams its N/8 shard of X and r exactly once (memory-bound).
Rows are packed 16-per-partition so each DMA descriptor moves 8 KiB.
Per 128-row group a single bf16 PE matmul accumulates

    psum[128, 257] += r_grp^T @ [x_bf16 | x.^2_bf16 | ones]

into one PSUM bank (fp32 accumulate).  bf16 inputs are safe: the loss is
linear in r and the round-to-nearest input errors (~2^-9 relative, zero
mean) cancel statistically over the 16M-product sums (~1e-6 net).
ScalarE squares X into the comb tile, VectorE casts X and r to bf16, and a
tiny tail contracts psum against Mext = [-2*mus | 1 | mu2] to a per-core
scalar.  Host sums the 8 partials.
"""

from contextlib import ExitStack

import numpy as np

import concourse.bass as bass
import concourse.tile as tile
from concourse import bacc, mybir
from concourse.bass_utils import run_bass_kernel_spmd

N, M, D = 131072, 128, 128
NCORES = 8
NSHARD = N // NCORES          # 16384 rows per core
P = 128                       # SBUF partitions
G = 16                        # rows packed per partition (8 KiB DMA runs)
ITERS = NSHARD // (P * G)     # 8
W = 2 * D + 1                 # moving cols per group: [x(128) | x^2(128) | 1]

F32 = mybir.dt.float32
BF16 = mybir.dt.bfloat16
_cache: dict[str, object] = {}


def _build():
    nc = bacc.Bacc(
        "TRN2",
        target_bir_lowering=False,
        debug=False,
        num_devices=NCORES,
    )

    x_d = nc.dram_tensor("X", [NSHARD, D], F32, kind="ExternalInput").ap()
    r_d = nc.dram_tensor("r", [NSHARD, M], F32, kind="ExternalInput").ap()
    mus_d = nc.dram_tensor("mus", [M, D], F32, kind="ExternalInput").ap()
    out_d = nc.dram_tensor("out", [1, 1], F32, kind="ExternalOutput").ap()

    # row = (i*P + p)*G + g  ->  AP dims [i, p, g, d]
    x_r = x_d.rearrange("(i p g) d -> i p g d", p=P, g=G)
    r_r = r_d.rearrange("(i p g) m -> i p g m", p=P, g=G)

    with tile.TileContext(nc) as tc, ExitStack() as ctx:
        singles = ctx.enter_context(tc.tile_pool(name="singles", bufs=1))
        big = ctx.enter_context(tc.tile_pool(name="big", bufs=3))
        psum_pool = ctx.enter_context(tc.tile_pool(name="psum", bufs=1, space="PSUM"))

        # Mext = [-2*mus | 1 | mu2], contracted against psum at the end.
        mus_t = singles.tile([M, D], F32)
        nc.sync.dma_start(mus_t[:], mus_d[:])
        mext = singles.tile([M, W], F32)
        nc.scalar.mul(mext[:, 0:D], mus_t[:], -2.0)
        nc.vector.memset(mext[:, D : 2 * D], 1.0)
        mus_sq = singles.tile([M, D], F32)
        nc.vector.tensor_mul(mus_sq[:], mus_t[:], mus_t[:])
        nc.vector.reduce_sum(
            mext[:, 2 * D : 2 * D + 1], mus_sq[:], axis=mybir.AxisListType.X
        )

        ones_col = singles.tile([P, 1], F32)
        nc.vector.memset(ones_col[:], 1.0)

        psum_acc = psum_pool.tile([M, W], F32)

        for t in range(ITERS):
            x_f32 = big.tile([P, G, D], F32, tag="x_f32")
            r_f32 = big.tile([P, G, M], F32, tag="r_f32")
            comb = big.tile([P, G, W], BF16, tag="comb")
            r_bf = big.tile([P, G, M], BF16, tag="r_bf")

            nc.sync.dma_start(x_f32[:], x_r[t])
            nc.sync.dma_start(r_f32[:], r_r[t])

            nc.vector.tensor_copy(comb[:, :, 0:D], x_f32[:])
            nc.scalar.activation(
                comb[:, :, D : 2 * D], x_f32[:], mybir.ActivationFunctionType.Square
            )
            nc.vector.tensor_copy(r_bf[:], r_f32[:])
            nc.vector.memset(comb[:, :, 2 * D : 2 * D + 1], 1.0)

            for g in range(G):
                idx = t * G + g
                nc.tensor.matmul(
                    psum_acc[:],
                    r_bf[:, g, :],
                    comb[:, g, :],
                    start=(idx == 0),
                    stop=(idx == ITERS * G - 1),
                )

        # Tail: scalar partial = sum(psum_acc * Mext)
        prod = singles.tile([M, W], F32)
        nc.vector.tensor_mul(prod[:], psum_acc[:], mext[:])
        u = singles.tile([M, 1], F32)
        nc.vector.reduce_sum(u[:], prod[:], axis=mybir.AxisListType.X)
        psum_s = psum_pool.tile([1, 1], F32)
        nc.tensor.matmul(psum_s[:], u[:], ones_col[:], start=True, stop=True)
        res = singles.tile([1, 1], F32)
        nc.vector.tensor_copy(res[:], psum_s[:])
        nc.sync.dma_start(out_d[:], res[:])

    nc.compile()
    return nc


def _get_nc():
    if "nc" not in _cache:
        _cache["nc"] = _build()
    return _cache["nc"]


def _run(X, r, mus, **spmd_kwargs):
    X = np.ascontiguousarray(np.asarray(X, dtype=np.float32))
    r = np.ascontiguousarray(np.asarray(r, dtype=np.float32))
    mus = np.ascontiguousarray(np.asarray(mus, dtype=np.float32))
    assert X.shape == (N, D) and r.shape == (N, M) and mus.shape == (M, D)

    nc = _get_nc()
    in_maps = [
        {
            "X": X[c * NSHARD : (c + 1) * NSHARD],
            "r": r[c * NSHARD : (c + 1) * NSHARD],
            "mus": mus,
        }
        for c in range(NCORES)
    ]
    return run_bass_kernel_spmd(nc, in_maps, core_ids=list(range(NCORES)), **spmd_kwargs)


def kernel(X, r, mus):
    out = _run(X, r, mus)
    total = sum(float(out.results[c]["out"][0, 0]) for c in range(NCORES))
    return np.float32(total / N)
